# revision 36
# baseline (speedup 1.0000x reference)
"""GQA attention kernel for 8 TRN2 NeuronCores (tensor-parallel over heads).

Problem: B=2, S=2048, D=2048, HQ=32, HKV=8, HD=64, ALiBi + additive mask,
softmax, out-projection.  Each core owns 4 q-heads (= 1 kv head); each core
computes a full-shape partial of the output (its heads' contribution through
wo), and the host sums the 8 partials.

Layout strategy (per core):
  - all matmuls in float32r (TF32-like, 1 cycle/row at N>=256)
  - logits computed TRANSPOSED: logitsT[n, m] = kaug.T @ qaug with the
    contraction dim augmented by 2 rows that add alibi slope*(n-m) and a
    per-query stabilizer -c[m] for free:
       kaug = [kT(64); n; 1]            (shared by all 4 heads)
       qaug_h = [qT_h(64); slope_h; -slope_h*m - c_h[m]]
  - PT = exp(logitsT) ; AV matmul uses vaug = [v | ones] so the ones column
    accumulates the softmax denominators in psum row 64.
  - normalization folded into the OT eviction (DVE multiply by broadcast
    reciprocal), odd heads DMA-shifted to partitions 64:127 so the
    o-projection reads one contiguous [128, m] stationary per head-pair.
  - causal masks: dead logit tiles are skipped entirely; diagonal-crossing
    tiles get one of ceil(MC/128) precomputed [128, MC] additive patterns.
"""

import os
import sys

sys.path.insert(0, "/opt/trn_rl_repo")

import numpy as np

NEG = -1e9


# ---------------------------------------------------------------------------
# device program builder
# ---------------------------------------------------------------------------

def build_program(cfg):
    import concourse.bass as bass  # noqa: F401
    import concourse.mybir as mybir
    import concourse.tile as tile
    from concourse import bacc

    f32 = mybir.dt.float32
    f32r = mybir.dt.float32r

    B, S, D = cfg["B"], cfg["S"], cfg["D"]
    HLOC, HD = cfg["HLOC"], cfg["HD"]
    MC = cfg["MC"]                    # m-chunk (<= 512, psum bank)
    MPAIR = 2 * MC                    # exp / AV / normalize granularity
    causal = cfg["causal"]
    generic_mask = cfg["generic_mask"]

    DQ = HLOC * HD                    # local q dims (256)
    NKT = D // 128                    # contraction k-tiles for projections
    NNT = S // 128                    # n-tiles (keys)
    NMC = S // MC                     # m-chunks per b
    NPAIR = S // MPAIR                # m-pairs per b
    NHP = HLOC // 2                   # head pairs
    NPAT = MC // 128                  # diagonal mask patterns
    NEC = D // MC                     # out-proj e-chunks
    NMT = S // 128                    # out-proj m-tiles

    nc = bacc.Bacc("TRN2", target_bir_lowering=False, debug=False)

    xT_d = nc.dram_tensor("xT", [D, B, S], f32, kind="ExternalInput")
    wq_d = nc.dram_tensor("wqT", [D, DQ], f32, kind="ExternalInput")
    wkv_d = nc.dram_tensor("wkvT", [D, 2 * HD], f32, kind="ExternalInput")
    wo_d = nc.dram_tensor("woT", [DQ, D], f32, kind="ExternalInput")
    kaug_d = nc.dram_tensor("kaug_ext", [2, S], f32, kind="ExternalInput")
    qaug_d = nc.dram_tensor("qaug_ext", [HLOC, 2, S], f32, kind="ExternalInput")
    ident_d = nc.dram_tensor("ident", [64, 64], f32, kind="ExternalInput")
    if causal:
        mpat_d = nc.dram_tensor("maskpat", [128, 128], f32, kind="ExternalInput")
    if generic_mask:
        maskT_d = nc.dram_tensor("maskT", [S, S], f32, kind="ExternalInput")
    out_d = nc.dram_tensor("out", [B, S, D], f32, kind="ExternalOutput")
    debug = cfg.get("debug", False)
    if debug:
        dbg = {}
        for nm, shape in [("dbg_qaug0", [66, S]), ("dbg_kaug", [66, S]),
                          ("dbg_vaug", [128, (S // 128) * (HD + 1)]),
                          ("dbg_otu", [65, 2 * MC]), ("dbg_rbc", [128, 2 * MC]),
                          ("dbg_OT0", [128, (HLOC // 2) * 2 * MC])]:
            dbg[nm] = nc.dram_tensor(nm, shape, f32, kind="ExternalOutput")

    def live(nt, mc):
        """is logitsT tile (keys nt*128.., queries mc*MC..) not fully masked"""
        if not causal:
            return True
        return nt * 128 <= mc * MC + MC - 1

    def crossing(nt, mc):
        """does the tile cross the causal diagonal (needs mask pattern)"""
        if not causal:
            return False
        return live(nt, mc) and nt * 128 + 127 > mc * MC

    with tile.TileContext(nc) as tc:
        with tc.tile_pool(name="res", bufs=1) as res, \
             tc.tile_pool(name="sbp", bufs=3) as sbp, \
             tc.tile_pool(name="ps", bufs=2, space="PSUM") as ps:

            # ---- resident tiles ------------------------------------------
            wq_sb = res.tile([128, NKT, DQ], f32r, tag="wq")
            half = NKT // 2
            nc.sync.dma_start(
                wq_sb[:, 0:half, :],
                wq_d.ap()[0:half * 128, :]
                .rearrange("(kt p) q -> p kt q", p=128).bitcast(f32r))
            wkv_sb = res.tile([128, NKT, 2 * HD], f32r, tag="wkv")
            nc.sync.dma_start(
                wkv_sb[:], wkv_d.ap()[:].rearrange("(kt p) q -> p kt q", p=128).bitcast(f32r))
            nc.sync.dma_start(
                wq_sb[:, half:, :],
                wq_d.ap()[half * 128:, :]
                .rearrange("(kt p) q -> p kt q", p=128).bitcast(f32r))
            wo_sb = res.tile([128, NHP, D], f32r, tag="wo")
            ident_sb = res.tile([64, 64], f32, tag="ident")
            nc.sync.dma_start(ident_sb[:], ident_d.ap()[:])
            if causal:
                mpat_sb = res.tile([128, 128], f32, tag="mpat")
                nc.sync.dma_start(mpat_sb[:], mpat_d.ap()[:])

            kaug = res.tile([66, S], f32r, tag="kaug")
            nc.sync.dma_start(kaug[64:66, :], kaug_d.ap()[:].bitcast(f32r))
            qaug = [res.tile([66, S], f32r, tag=f"qaug{h}", name=f"qaug{h}")
                    for h in range(HLOC)]
            for h in range(HLOC):
                nc.sync.dma_start(qaug[h][64:66, :], qaug_d.ap()[h].bitcast(f32r))
            vaug = res.tile([128, NNT, HD + 1], f32r, tag="vaug")
            nc.vector.memset(vaug[:].bitcast(f32), 1.0)
            vt_sb = res.tile([64, S], f32, tag="vt")
            OT_sb = [res.tile([128, NHP, MPAIR], f32r, tag=f"OT{p}", name=f"OT{p}")
                     for p in range(NPAIR)]

            for _rep in range(cfg.get("reps", 1)):
              for b in range(B):
                # ---- projections: qT, kT, vT for this b ------------------
                for mc in range(NMC):
                    mco = mc * MC
                    qp = ps.tile([128, NHP * MC], f32, tag="qk")
                    kvp = ps.tile([128, MC], f32, tag="av")
                    KQ = 4  # k-tiles per xt DMA
                    for ktq in range(NKT // KQ):
                        xt = sbp.tile([128, KQ, MC], f32r, tag="xt", bufs=4)
                        nc.sync.dma_start(
                            xt[:], xT_d.ap()[ktq * KQ * 128:(ktq + 1) * KQ * 128,
                                             b, mco:mco + MC]
                            .rearrange("(k p) m -> p k m", p=128).bitcast(f32r))
                        for kq in range(KQ):
                            kt = ktq * KQ + kq
                            st, sp = (kt == 0), (kt == NKT - 1)
                            for hp in range(NHP):
                                nc.tensor.matmul(
                                    qp[:, hp * MC:(hp + 1) * MC],
                                    wq_sb[:, kt, hp * 128:(hp + 1) * 128],
                                    xt[:, kq], start=st, stop=sp)
                            nc.tensor.matmul(kvp[:], wkv_sb[:, kt, :], xt[:, kq],
                                             start=st, stop=sp)
                    # evictions
                    for hp in range(NHP):
                        # even head of the pair: psum rows 0:64 -> qaug rows 0:64
                        nc.vector.tensor_copy(qaug[2 * hp][0:64, mco:mco + MC],
                                              qp[0:64, hp * MC:(hp + 1) * MC])
                        # odd head: rows 64:128, engine-copy then DMA shift
                        qtmp = sbp.tile([128, MC], f32r, tag="tmp", bufs=2)
                        nc.vector.tensor_copy(qtmp[64:128, :],
                                               qp[64:128, hp * MC:(hp + 1) * MC])
                        nc.sync.dma_start(qaug[2 * hp + 1][0:64, mco:mco + MC],
                                          qtmp[64:128, :])
                    nc.vector.tensor_copy(kaug[0:64, mco:mco + MC], kvp[0:64, :])
                    vtmp = sbp.tile([128, MC], f32, tag="tmp", bufs=2)
                    nc.vector.tensor_copy(vtmp[64:128, :], kvp[64:128, :])
                    nc.sync.dma_start(vt_sb[0:64, mco:mco + MC], vtmp[64:128, :])

                # ---- transpose vT -> v (vaug) ----------------------------
                # groups of 8 n-tiles per psum tile
                for g in range((NNT + 7) // 8):
                    nts = range(g * 8, min((g + 1) * 8, NNT))
                    vtp = ps.tile([128, 512], f32, tag="av")
                    for j, nt in enumerate(nts):
                        nc.tensor.transpose(
                            vtp[:, j * 64:(j + 1) * 64],
                            vt_sb[0:64, nt * 128:(nt + 1) * 128], ident_sb[:])
                    nc.vector.tensor_copy(vaug[:, nts.start:nts.stop, 0:HD],
                                            vtp[:, 0:64 * len(nts)].rearrange(
                                                "p (t d) -> p t d", d=64))

                if debug and b == 0:
                    nc.sync.dma_start(dbg["dbg_qaug0"].ap()[:],
                                      qaug[0][:].bitcast(f32))
                    nc.sync.dma_start(dbg["dbg_kaug"].ap()[:],
                                      kaug[:].bitcast(f32))
                    nc.sync.dma_start(
                        dbg["dbg_vaug"].ap()[:],
                        vaug[:].rearrange("p a b -> p (a b)").bitcast(f32))

                # ---- attention (pair-outer) + interleaved out-proj -------
                for pair in range(NPAIR):
                    po = pair * MPAIR
                    for h in range(HLOC):
                        hp, odd = h // 2, h % 2
                        av = [ps.tile([128, MC], f32, tag="av", name=f"av{c}")
                              for c in range(2)]
                        nlive = [nt for nt in range(NNT)
                                 if live(nt, 2 * pair) or live(nt, 2 * pair + 1)]
                        for nt in nlive:
                            qk = ps.tile([128, MPAIR], f32, tag="qk")
                            pt_t = sbp.tile([128, MPAIR], f32r, tag="pt", bufs=5)
                            ch_live = [c for c in range(2) if live(nt, 2 * pair + c)]
                            offs = {}
                            for c in ch_live:
                                mc = 2 * pair + c
                                # cols [0, o) of this chunk are fully masked
                                o = max(0, nt * 128 - mc * MC) if causal else 0
                                offs[c] = o
                                lo = c * MC + o
                                nc.tensor.matmul(
                                    qk[:, lo:(c + 1) * MC],
                                    kaug[:, nt * 128:(nt + 1) * 128],
                                    qaug[h][:, mc * MC + o:(mc + 1) * MC],
                                    start=True, stop=True)
                                if generic_mask:
                                    mtile = sbp.tile([128, MC], f32, tag="mt")
                                    nc.sync.dma_start(
                                        mtile[:],
                                        maskT_d.ap()[nt * 128:(nt + 1) * 128,
                                                     mc * MC:(mc + 1) * MC])
                                    nc.vector.tensor_add(
                                        qk[:, c * MC:(c + 1) * MC],
                                        qk[:, c * MC:(c + 1) * MC], mtile[:])
                                elif crossing(nt, mc):
                                    # triangular band on cols [o, o+128)
                                    nc.vector.tensor_add(
                                        qk[:, lo:lo + 128],
                                        qk[:, lo:lo + 128], mpat_sb[:])
                            c0, c1 = ch_live[0], ch_live[-1] + 1
                            o0 = offs[c0]
                            if o0:
                                nc.vector.memset(pt_t[:, c0 * MC:c0 * MC + o0].bitcast(f32), 0.0)
                            nc.scalar.activation(
                                pt_t[:, c0 * MC + o0:c1 * MC],
                                qk[:, c0 * MC + o0:c1 * MC],
                                mybir.ActivationFunctionType.Exp)
                            for c in ch_live:
                                mc = 2 * pair + c
                                last_nt = (mc * MC + MC - 1) // 128 if causal else NNT - 1
                                nc.tensor.matmul(
                                    av[c][0:HD + 1, :],
                                    vaug[:, nt, :], pt_t[:, c * MC:(c + 1) * MC],
                                    start=(nt == 0), stop=(nt == last_nt))
                        # evict unnormalized OT+sums immediately (frees psum)
                        otu = sbp.tile([65, MPAIR], f32, tag="otu", bufs=2)
                        for c in range(2):
                            nc.vector.tensor_copy(otu[0:65, c * MC:(c + 1) * MC],
                                                  av[c][0:HD + 1, :])
                        srow = sbp.tile([1, MPAIR], f32, tag="srow", bufs=2)
                        nc.sync.dma_start(srow[0:1, :], otu[64:65, :])
                        rbc = sbp.tile([128, MPAIR], f32, tag="rbc", bufs=2)
                        nc.gpsimd.partition_broadcast(rbc[:], srow[0:1, :])
                        nc.vector.reciprocal(rbc[:], rbc[:])
                        if debug and b == 0 and pair == 0 and h == 0:
                            nc.sync.dma_start(dbg["dbg_otu"].ap()[:], otu[:])
                            nc.sync.dma_start(dbg["dbg_rbc"].ap()[:], rbc[:])
                        if not odd:
                            nc.vector.tensor_mul(
                                OT_sb[pair][0:64, hp, :],
                                otu[0:64, :], rbc[0:64, :])
                        else:
                            nc.sync.dma_start(OT_sb[pair][64:128, hp, :],
                                              otu[0:64, :].bitcast(f32r))
                            nc.vector.tensor_mul(
                                OT_sb[pair][64:128, hp, :],
                                OT_sb[pair][64:128, hp, :], rbc[64:128, :])
                    if debug and b == 0 and pair == 0:
                        nc.sync.dma_start(
                            dbg["dbg_OT0"].ap()[:],
                            OT_sb[0][:].rearrange("p a b -> p (a b)").bitcast(f32))
                    # ---- out-projection for this pair's m-tiles ----------
                    if b == 0 and pair == 0:
                        nc.sync.dma_start(
                            wo_sb[:],
                            wo_d.ap()[:].rearrange("(hp p) e -> p hp e",
                                                   p=128).bitcast(f32r))
                    for mtl in range(MPAIR // 128):
                        mt = pair * (MPAIR // 128) + mtl
                        ob = sbp.tile([128, D], f32, tag="ob", bufs=2)
                        for ec in range(NEC):
                            op = ps.tile([128, MC], f32, tag="pp")
                            for hp in range(NHP):
                                nc.tensor.matmul(
                                    op[:],
                                    OT_sb[pair][:, hp, mtl * 128:(mtl + 1) * 128],
                                    wo_sb[:, hp, ec * MC:(ec + 1) * MC],
                                    start=(hp == 0), stop=(hp == NHP - 1))
                            nc.vector.tensor_copy(ob[:, ec * MC:(ec + 1) * MC],
                                                  op[:])
                        nc.sync.dma_start(
                            out_d.ap()[b, mt * 128:(mt + 1) * 128, :], ob[:])

    nc.compile()
    return nc


# ---------------------------------------------------------------------------
# host side
# ---------------------------------------------------------------------------

def _analyze_mask(mask2d, S):
    """classify mask; return (causal, zeros, n_lo, n_hi)"""
    masked = mask2d < -1e8
    if not masked.any():
        return False, True, np.zeros(S, np.int64), np.full(S, S - 1, np.int64)
    tri = np.triu(np.ones((S, S), bool), 1)
    if (masked == tri).all() and (mask2d[~masked] == 0).all():
        return True, False, np.zeros(S, np.int64), np.arange(S)
    allowed = ~masked
    # guard fully-masked rows (keep index 0; softmax row is garbage anyway)
    any_allowed = allowed.any(axis=1)
    idx = np.arange(S)[None, :]
    n_hi = np.where(any_allowed, np.where(allowed, idx, -1).max(axis=1), 0)
    n_lo = np.where(any_allowed, np.where(allowed, idx, S).min(axis=1), 0)
    return False, False, n_lo, n_hi


def _make_inputs_for_core(core, x, wq, wk, wv, wo, slopes, mask, cfg):
    B, S, D, HLOC, HD = cfg["B"], cfg["S"], cfg["D"], cfg["HLOC"], cfg["HD"]
    MC = cfg["MC"]
    h0 = core * HLOC
    kv = core  # one kv head per core
    scale = 1.0 / np.sqrt(HD)

    xT = np.ascontiguousarray(x.transpose(2, 0, 1))                 # [D,B,S]
    wqT = np.ascontiguousarray((wq[h0 * HD:(h0 + HLOC) * HD] * scale).T)
    wkvT = np.ascontiguousarray(
        np.concatenate([wk[kv * HD:(kv + 1) * HD], wv[kv * HD:(kv + 1) * HD]],
                       axis=0).T)                                   # [D,128]
    woT = np.ascontiguousarray(wo[:, h0 * HD:(h0 + HLOC) * HD].T)   # [DQ,D]

    n = np.arange(S, dtype=np.float32)
    kaug_ext = np.stack([n, np.ones(S, np.float32)])                # [2,S]

    qaug_ext = np.zeros((HLOC, 2, S), np.float32)
    for i in range(HLOC):
        sl = float(slopes[h0 + i])
        # stabilizer c[m] = max over allowed n of slope*(n-m), clipped >= 0
        c = np.maximum(0.0, np.maximum(sl * (cfg["n_hi"] - n),
                                       sl * (cfg["n_lo"] - n)))
        qaug_ext[i, 0, :] = sl
        qaug_ext[i, 1, :] = -sl * n - c

    ident = np.eye(64, dtype=np.float32)

    ins = {"xT": xT, "wqT": wqT, "wkvT": wkvT, "woT": woT,
           "kaug_ext": kaug_ext, "qaug_ext": qaug_ext, "ident": ident}
    if cfg["causal"]:
        ii = np.arange(128)[:, None]
        jj = np.arange(128)[None, :]
        ins["maskpat"] = np.where(ii > jj, NEG, 0.0).astype(np.float32)
    if cfg["generic_mask"]:
        ins["maskT"] = np.ascontiguousarray(mask[0, 0].T)
    return ins


def _host_reference_partial(core, inputs, cfg):
    """numpy emulation of one core's partial (for testing the builder)"""
    x, wq, wk, wv, wo = (inputs[k] for k in ("x", "wq", "wk", "wv", "wo"))
    slopes, mask = inputs["slopes"], inputs["mask"]
    B, S, HLOC, HD = cfg["B"], cfg["S"], cfg["HLOC"], cfg["HD"]
    h0, kvh = core * HLOC, core
    q = (x @ wq.T)[..., h0 * HD:(h0 + HLOC) * HD]
    k = (x @ wk.T)[..., kvh * HD:(kvh + 1) * HD]
    v = (x @ wv.T)[..., kvh * HD:(kvh + 1) * HD]
    out = np.zeros_like(x)
    rel = (np.arange(S)[None, :] - np.arange(S)[:, None]).astype(np.float32)
    for h in range(HLOC):
        qh = q[..., h * HD:(h + 1) * HD] / np.sqrt(HD)
        lg = np.einsum('bmd,bnd->bmn', qh, k)
        lg += slopes[h0 + h] * rel[None] + mask[0]
        lg -= lg.max(axis=-1, keepdims=True)
        p = np.exp(lg)
        p /= p.sum(axis=-1, keepdims=True)
        oh = np.einsum('bmn,bnd->bmd', p, v)
        out += oh @ wo[:, (h0 + h) * HD:(h0 + h + 1) * HD].T
    return out


def kernel(x, wq, wk, wv, wo, slopes, mask, _debug_sim=False):
    from concourse.bass_utils import run_bass_kernel_spmd

    x = np.asarray(x, dtype=np.float32)
    wq = np.asarray(wq, dtype=np.float32)
    wk = np.asarray(wk, dtype=np.float32)
    wv = np.asarray(wv, dtype=np.float32)
    wo = np.asarray(wo, dtype=np.float32)
    slopes = np.asarray(slopes, dtype=np.float32)
    mask = np.asarray(mask, dtype=np.float32)

    B, S, D = x.shape
    HQ = 32
    HD = D // HQ
    n_cores = 8
    HLOC = HQ // n_cores

    causal, zeros, n_lo, n_hi = _analyze_mask(mask[0, 0], S)
    cfg = dict(B=B, S=S, D=D, HLOC=HLOC, HD=HD, MC=512,
               causal=causal, generic_mask=not (causal or zeros),
               n_lo=n_lo, n_hi=n_hi)

    nc = build_program(cfg)
    in_maps = [_make_inputs_for_core(c, x, wq, wk, wv, wo, slopes, mask, cfg)
               for c in range(n_cores)]
    res = run_bass_kernel_spmd(nc, in_maps, core_ids=list(range(n_cores)))
    out = np.zeros((B, S, D), np.float32)
    for c in range(n_cores):
        out += res.results[c]["out"]
    return out


if __name__ == "__main__":
    # quick self-test with a tiny config through CoreSim
    pass


# revision 37
# speedup vs baseline: 1.5711x; 1.5711x over previous
"""GQA attention kernel for 8 TRN2 NeuronCores (tensor-parallel over heads).

Problem: B=2, S=2048, D=2048, HQ=32, HKV=8, HD=64, ALiBi + additive mask,
softmax, out-projection.  Each core owns 4 q-heads (= 1 kv head); each core
computes a full-shape partial of the output (its heads' contribution through
wo), and the host sums the 8 partials.

Layout strategy (per core):
  - all matmuls in float32r (TF32-like, 1 cycle/row at N>=256)
  - logits computed TRANSPOSED: logitsT[n, m] = kaug.T @ qaug with the
    contraction dim augmented by 2 rows that add alibi slope*(n-m) and a
    per-query stabilizer -c[m] for free:
       kaug = [kT(64); n; 1]            (shared by all 4 heads)
       qaug_h = [qT_h(64); slope_h; -slope_h*m - c_h[m]]
  - PT = exp(logitsT) ; AV matmul uses vaug = [v | ones] so the ones column
    accumulates the softmax denominators in psum row 64.
  - normalization folded into the OT eviction (DVE multiply by broadcast
    reciprocal), odd heads DMA-shifted to partitions 64:127 so the
    o-projection reads one contiguous [128, m] stationary per head-pair.
  - causal masks: dead logit tiles are skipped entirely; diagonal-crossing
    tiles get one of ceil(MC/128) precomputed [128, MC] additive patterns.
"""

import os
import sys

sys.path.insert(0, "/opt/trn_rl_repo")

import numpy as np

NEG = -1e9


# ---------------------------------------------------------------------------
# device program builder
# ---------------------------------------------------------------------------

def build_program(cfg):
    import concourse.bass as bass  # noqa: F401
    import concourse.mybir as mybir
    import concourse.tile as tile
    from concourse import bacc

    f32 = mybir.dt.float32
    f32r = mybir.dt.float32r

    B, S, D = cfg["B"], cfg["S"], cfg["D"]
    HLOC, HD = cfg["HLOC"], cfg["HD"]
    MC = cfg["MC"]                    # m-chunk (<= 512, psum bank)
    MPAIR = 2 * MC                    # exp / AV / normalize granularity
    causal = cfg["causal"]
    generic_mask = cfg["generic_mask"]

    DQ = HLOC * HD                    # local q dims (256)
    NKT = D // 128                    # contraction k-tiles for projections
    NNT = S // 128                    # n-tiles (keys)
    NMC = S // MC                     # m-chunks per b
    NPAIR = S // MPAIR                # m-pairs per b
    NHP = HLOC // 2                   # head pairs
    NPAT = MC // 128                  # diagonal mask patterns
    NEC = D // MC                     # out-proj e-chunks
    NMT = S // 128                    # out-proj m-tiles

    nc = bacc.Bacc("TRN2", target_bir_lowering=False, debug=False)

    xT_d = nc.dram_tensor("xT", [D, B, S], f32, kind="ExternalInput")
    wq_d = nc.dram_tensor("wqT", [D, DQ], f32, kind="ExternalInput")
    wkv_d = nc.dram_tensor("wkvT", [D, 2 * HD], f32, kind="ExternalInput")
    wo_d = nc.dram_tensor("woT", [DQ, D], f32, kind="ExternalInput")
    kaug_d = nc.dram_tensor("kaug_ext", [2, S], f32, kind="ExternalInput")
    qaug_d = nc.dram_tensor("qaug_ext", [HLOC, 2, S], f32, kind="ExternalInput")
    ident_d = nc.dram_tensor("ident", [64, 64], f32, kind="ExternalInput")
    if causal:
        mpat_d = nc.dram_tensor("maskpat", [128, 128], f32, kind="ExternalInput")
    if generic_mask:
        maskT_d = nc.dram_tensor("maskT", [S, S], f32, kind="ExternalInput")
    out_d = nc.dram_tensor("out", [B, S, D], f32, kind="ExternalOutput")
    debug = cfg.get("debug", False)
    if debug:
        dbg = {}
        for nm, shape in [("dbg_qaug0", [66, S]), ("dbg_kaug", [66, S]),
                          ("dbg_vaug", [128, (S // 128) * (HD + 1)]),
                          ("dbg_otu", [65, 2 * MC]), ("dbg_rbc", [128, 2 * MC]),
                          ("dbg_OT0", [128, (HLOC // 2) * 2 * MC])]:
            dbg[nm] = nc.dram_tensor(nm, shape, f32, kind="ExternalOutput")

    def live(nt, mc):
        """is logitsT tile (keys nt*128.., queries mc*MC..) not fully masked"""
        if not causal:
            return True
        return nt * 128 <= mc * MC + MC - 1

    def crossing(nt, mc):
        """does the tile cross the causal diagonal (needs mask pattern)"""
        if not causal:
            return False
        return live(nt, mc) and nt * 128 + 127 > mc * MC

    with tile.TileContext(nc) as tc:
        with tc.tile_pool(name="res", bufs=1) as res, \
             tc.tile_pool(name="sbp", bufs=3) as sbp, \
             tc.tile_pool(name="ps", bufs=2, space="PSUM") as ps:

            # ---- resident tiles ------------------------------------------
            wq_sb = res.tile([128, NKT, DQ], f32r, tag="wq")
            wkv_sb = res.tile([128, NKT, 2 * HD], f32r, tag="wkv")
            # interleave quarter-loads of wq/wkv so the first k-tiles land fast
            qtr = NKT // 4
            for qi in range(4):
                sl = slice(qi * qtr * 128, (qi + 1) * qtr * 128)
                nc.sync.dma_start(
                    wq_sb[:, qi * qtr:(qi + 1) * qtr, :],
                    wq_d.ap()[sl, :]
                    .rearrange("(kt p) q -> p kt q", p=128).bitcast(f32r))
                nc.sync.dma_start(
                    wkv_sb[:, qi * qtr:(qi + 1) * qtr, :],
                    wkv_d.ap()[sl, :]
                    .rearrange("(kt p) q -> p kt q", p=128).bitcast(f32r))
            wo_sb = res.tile([128, NHP, D], f32r, tag="wo")
            ident_sb = res.tile([64, 64], f32, tag="ident")
            nc.sync.dma_start(ident_sb[:], ident_d.ap()[:])
            if causal:
                mpat_sb = res.tile([128, 128], f32, tag="mpat")
                nc.sync.dma_start(mpat_sb[:], mpat_d.ap()[:])

            kaug = res.tile([66, S], f32r, tag="kaug")
            nc.sync.dma_start(kaug[64:66, :], kaug_d.ap()[:].bitcast(f32r))
            qaug = [res.tile([66, S], f32r, tag=f"qaug{h}", name=f"qaug{h}")
                    for h in range(HLOC)]
            for h in range(HLOC):
                nc.sync.dma_start(qaug[h][64:66, :], qaug_d.ap()[h].bitcast(f32r))
            vaug = res.tile([128, NNT, HD + 1], f32r, tag="vaug")
            nc.vector.memset(vaug[:].bitcast(f32), 1.0)
            vt_sb = res.tile([64, S], f32, tag="vt")
            OT_sb = [res.tile([128, NHP, MPAIR], f32r, tag=f"OT{p}", name=f"OT{p}")
                     for p in range(NPAIR)]

            for _rep in range(cfg.get("reps", 1)):
              for b in range(B):
                # ---- projections: qT, kT, vT for this b ------------------
                for mc in range(NMC):
                    mco = mc * MC
                    qp = ps.tile([128, NHP * MC], f32, tag="qk")
                    kvp = ps.tile([128, MC], f32, tag="av")
                    KQ = 4  # k-tiles per xt DMA
                    for ktq in range(NKT // KQ):
                        xt = sbp.tile([128, KQ, MC], f32r, tag="xt", bufs=4)
                        nc.sync.dma_start(
                            xt[:], xT_d.ap()[ktq * KQ * 128:(ktq + 1) * KQ * 128,
                                             b, mco:mco + MC]
                            .rearrange("(k p) m -> p k m", p=128).bitcast(f32r))
                        for kq in range(KQ):
                            kt = ktq * KQ + kq
                            st, sp = (kt == 0), (kt == NKT - 1)
                            for hp in range(NHP):
                                nc.tensor.matmul(
                                    qp[:, hp * MC:(hp + 1) * MC],
                                    wq_sb[:, kt, hp * 128:(hp + 1) * 128],
                                    xt[:, kq], start=st, stop=sp)
                            nc.tensor.matmul(kvp[:], wkv_sb[:, kt, :], xt[:, kq],
                                             start=st, stop=sp)
                    # evictions
                    for hp in range(NHP):
                        # even head of the pair: psum rows 0:64 -> qaug rows 0:64
                        nc.vector.tensor_copy(qaug[2 * hp][0:64, mco:mco + MC],
                                              qp[0:64, hp * MC:(hp + 1) * MC])
                        # odd head: rows 64:128, engine-copy then DMA shift
                        qtmp = sbp.tile([128, MC], f32r, tag="tmp", bufs=2)
                        nc.vector.tensor_copy(qtmp[64:128, :],
                                               qp[64:128, hp * MC:(hp + 1) * MC])
                        nc.sync.dma_start(qaug[2 * hp + 1][0:64, mco:mco + MC],
                                          qtmp[64:128, :])
                    nc.vector.tensor_copy(kaug[0:64, mco:mco + MC], kvp[0:64, :])
                    vtmp = sbp.tile([128, MC], f32, tag="tmp", bufs=2)
                    nc.vector.tensor_copy(vtmp[64:128, :], kvp[64:128, :])
                    nc.sync.dma_start(vt_sb[0:64, mco:mco + MC], vtmp[64:128, :])

                # ---- transpose vT -> v (vaug) ----------------------------
                # groups of 8 n-tiles per psum tile
                for g in range((NNT + 7) // 8):
                    nts = range(g * 8, min((g + 1) * 8, NNT))
                    vtp = ps.tile([128, 512], f32, tag="av")
                    for j, nt in enumerate(nts):
                        nc.tensor.transpose(
                            vtp[:, j * 64:(j + 1) * 64],
                            vt_sb[0:64, nt * 128:(nt + 1) * 128], ident_sb[:])
                    nc.vector.tensor_copy(vaug[:, nts.start:nts.stop, 0:HD],
                                            vtp[:, 0:64 * len(nts)].rearrange(
                                                "p (t d) -> p t d", d=64))

                if debug and b == 0:
                    nc.sync.dma_start(dbg["dbg_qaug0"].ap()[:],
                                      qaug[0][:].bitcast(f32))
                    nc.sync.dma_start(dbg["dbg_kaug"].ap()[:],
                                      kaug[:].bitcast(f32))
                    nc.sync.dma_start(
                        dbg["dbg_vaug"].ap()[:],
                        vaug[:].rearrange("p a b -> p (a b)").bitcast(f32))

                # ---- attention (pair-outer) + interleaved out-proj -------
                for pair in range(NPAIR):
                    po = pair * MPAIR
                    for h in range(HLOC):
                        hp, odd = h // 2, h % 2
                        av = [ps.tile([128, MC], f32, tag="av", name=f"av{c}")
                              for c in range(2)]
                        nlive = [nt for nt in range(NNT)
                                 if live(nt, 2 * pair) or live(nt, 2 * pair + 1)]
                        for nt in nlive:
                            qk = ps.tile([128, MPAIR], f32, tag="qk")
                            pt_t = sbp.tile([128, MPAIR], f32r, tag="pt", bufs=5)
                            ch_live = [c for c in range(2) if live(nt, 2 * pair + c)]
                            offs = {}
                            for c in ch_live:
                                mc = 2 * pair + c
                                # cols [0, o) of this chunk are fully masked
                                o = max(0, nt * 128 - mc * MC) if causal else 0
                                offs[c] = o
                                lo = c * MC + o
                                nc.tensor.matmul(
                                    qk[:, lo:(c + 1) * MC],
                                    kaug[:, nt * 128:(nt + 1) * 128],
                                    qaug[h][:, mc * MC + o:(mc + 1) * MC],
                                    start=True, stop=True)
                                if generic_mask:
                                    mtile = sbp.tile([128, MC], f32, tag="mt")
                                    nc.sync.dma_start(
                                        mtile[:],
                                        maskT_d.ap()[nt * 128:(nt + 1) * 128,
                                                     mc * MC:(mc + 1) * MC])
                                    nc.vector.tensor_add(
                                        qk[:, c * MC:(c + 1) * MC],
                                        qk[:, c * MC:(c + 1) * MC], mtile[:])
                                elif crossing(nt, mc):
                                    # triangular band on cols [o, o+128)
                                    nc.vector.tensor_add(
                                        qk[:, lo:lo + 128],
                                        qk[:, lo:lo + 128], mpat_sb[:])
                            c0, c1 = ch_live[0], ch_live[-1] + 1
                            o0 = offs[c0]
                            if o0:
                                nc.vector.memset(pt_t[:, c0 * MC:c0 * MC + o0].bitcast(f32), 0.0)
                            nc.scalar.activation(
                                pt_t[:, c0 * MC + o0:c1 * MC],
                                qk[:, c0 * MC + o0:c1 * MC],
                                mybir.ActivationFunctionType.Exp)
                            for c in ch_live:
                                mc = 2 * pair + c
                                last_nt = (mc * MC + MC - 1) // 128 if causal else NNT - 1
                                nc.tensor.matmul(
                                    av[c][0:HD + 1, :],
                                    vaug[:, nt, :], pt_t[:, c * MC:(c + 1) * MC],
                                    start=(nt == 0), stop=(nt == last_nt))
                        # evict unnormalized OT+sums immediately (frees psum)
                        otu = sbp.tile([65, MPAIR], f32, tag="otu", bufs=2)
                        for c in range(2):
                            nc.vector.tensor_copy(otu[0:65, c * MC:(c + 1) * MC],
                                                  av[c][0:HD + 1, :])
                        srow = sbp.tile([1, MPAIR], f32, tag="srow", bufs=2)
                        nc.sync.dma_start(srow[0:1, :], otu[64:65, :])
                        rbc = sbp.tile([128, MPAIR], f32, tag="rbc", bufs=2)
                        nc.gpsimd.partition_broadcast(rbc[:], srow[0:1, :])
                        nc.vector.reciprocal(rbc[:], rbc[:])
                        if debug and b == 0 and pair == 0 and h == 0:
                            nc.sync.dma_start(dbg["dbg_otu"].ap()[:], otu[:])
                            nc.sync.dma_start(dbg["dbg_rbc"].ap()[:], rbc[:])
                        if not odd:
                            nc.vector.tensor_mul(
                                OT_sb[pair][0:64, hp, :],
                                otu[0:64, :], rbc[0:64, :])
                        else:
                            nc.sync.dma_start(OT_sb[pair][64:128, hp, :],
                                              otu[0:64, :].bitcast(f32r))
                            nc.vector.tensor_mul(
                                OT_sb[pair][64:128, hp, :],
                                OT_sb[pair][64:128, hp, :], rbc[64:128, :])
                    if debug and b == 0 and pair == 0:
                        nc.sync.dma_start(
                            dbg["dbg_OT0"].ap()[:],
                            OT_sb[0][:].rearrange("p a b -> p (a b)").bitcast(f32))
                    # ---- out-projection for this pair's m-tiles ----------
                    if b == 0 and pair == 0:
                        nc.sync.dma_start(
                            wo_sb[:],
                            wo_d.ap()[:].rearrange("(hp p) e -> p hp e",
                                                   p=128).bitcast(f32r))
                    for mtl in range(MPAIR // 128):
                        mt = pair * (MPAIR // 128) + mtl
                        ob = sbp.tile([128, D], f32, tag="ob", bufs=2)
                        for ec in range(NEC):
                            op = ps.tile([128, MC], f32, tag="pp")
                            for hp in range(NHP):
                                nc.tensor.matmul(
                                    op[:],
                                    OT_sb[pair][:, hp, mtl * 128:(mtl + 1) * 128],
                                    wo_sb[:, hp, ec * MC:(ec + 1) * MC],
                                    start=(hp == 0), stop=(hp == NHP - 1))
                            nc.vector.tensor_copy(ob[:, ec * MC:(ec + 1) * MC],
                                                  op[:])
                        nc.sync.dma_start(
                            out_d.ap()[b, mt * 128:(mt + 1) * 128, :], ob[:])

    nc.compile()
    return nc


# ---------------------------------------------------------------------------
# host side
# ---------------------------------------------------------------------------

def _analyze_mask(mask2d, S):
    """classify mask; return (causal, zeros, n_lo, n_hi)"""
    masked = mask2d < -1e8
    if not masked.any():
        return False, True, np.zeros(S, np.int64), np.full(S, S - 1, np.int64)
    tri = np.triu(np.ones((S, S), bool), 1)
    if (masked == tri).all() and (mask2d[~masked] == 0).all():
        return True, False, np.zeros(S, np.int64), np.arange(S)
    allowed = ~masked
    # guard fully-masked rows (keep index 0; softmax row is garbage anyway)
    any_allowed = allowed.any(axis=1)
    idx = np.arange(S)[None, :]
    n_hi = np.where(any_allowed, np.where(allowed, idx, -1).max(axis=1), 0)
    n_lo = np.where(any_allowed, np.where(allowed, idx, S).min(axis=1), 0)
    return False, False, n_lo, n_hi


def _make_inputs_for_core(core, x, wq, wk, wv, wo, slopes, mask, cfg):
    B, S, D, HLOC, HD = cfg["B"], cfg["S"], cfg["D"], cfg["HLOC"], cfg["HD"]
    MC = cfg["MC"]
    h0 = core * HLOC
    kv = core  # one kv head per core
    scale = 1.0 / np.sqrt(HD)

    xT = np.ascontiguousarray(x.transpose(2, 0, 1))                 # [D,B,S]
    wqT = np.ascontiguousarray((wq[h0 * HD:(h0 + HLOC) * HD] * scale).T)
    wkvT = np.ascontiguousarray(
        np.concatenate([wk[kv * HD:(kv + 1) * HD], wv[kv * HD:(kv + 1) * HD]],
                       axis=0).T)                                   # [D,128]
    woT = np.ascontiguousarray(wo[:, h0 * HD:(h0 + HLOC) * HD].T)   # [DQ,D]

    n = np.arange(S, dtype=np.float32)
    kaug_ext = np.stack([n, np.ones(S, np.float32)])                # [2,S]

    qaug_ext = np.zeros((HLOC, 2, S), np.float32)
    for i in range(HLOC):
        sl = float(slopes[h0 + i])
        # stabilizer c[m] = max over allowed n of slope*(n-m), clipped >= 0
        c = np.maximum(0.0, np.maximum(sl * (cfg["n_hi"] - n),
                                       sl * (cfg["n_lo"] - n)))
        qaug_ext[i, 0, :] = sl
        qaug_ext[i, 1, :] = -sl * n - c

    ident = np.eye(64, dtype=np.float32)

    ins = {"xT": xT, "wqT": wqT, "wkvT": wkvT, "woT": woT,
           "kaug_ext": kaug_ext, "qaug_ext": qaug_ext, "ident": ident}
    if cfg["causal"]:
        ii = np.arange(128)[:, None]
        jj = np.arange(128)[None, :]
        ins["maskpat"] = np.where(ii > jj, NEG, 0.0).astype(np.float32)
    if cfg["generic_mask"]:
        ins["maskT"] = np.ascontiguousarray(mask[0, 0].T)
    return ins


def _host_reference_partial(core, inputs, cfg):
    """numpy emulation of one core's partial (for testing the builder)"""
    x, wq, wk, wv, wo = (inputs[k] for k in ("x", "wq", "wk", "wv", "wo"))
    slopes, mask = inputs["slopes"], inputs["mask"]
    B, S, HLOC, HD = cfg["B"], cfg["S"], cfg["HLOC"], cfg["HD"]
    h0, kvh = core * HLOC, core
    q = (x @ wq.T)[..., h0 * HD:(h0 + HLOC) * HD]
    k = (x @ wk.T)[..., kvh * HD:(kvh + 1) * HD]
    v = (x @ wv.T)[..., kvh * HD:(kvh + 1) * HD]
    out = np.zeros_like(x)
    rel = (np.arange(S)[None, :] - np.arange(S)[:, None]).astype(np.float32)
    for h in range(HLOC):
        qh = q[..., h * HD:(h + 1) * HD] / np.sqrt(HD)
        lg = np.einsum('bmd,bnd->bmn', qh, k)
        lg += slopes[h0 + h] * rel[None] + mask[0]
        lg -= lg.max(axis=-1, keepdims=True)
        p = np.exp(lg)
        p /= p.sum(axis=-1, keepdims=True)
        oh = np.einsum('bmn,bnd->bmd', p, v)
        out += oh @ wo[:, (h0 + h) * HD:(h0 + h + 1) * HD].T
    return out


def kernel(x, wq, wk, wv, wo, slopes, mask, _debug_sim=False):
    from concourse.bass_utils import run_bass_kernel_spmd

    x = np.asarray(x, dtype=np.float32)
    wq = np.asarray(wq, dtype=np.float32)
    wk = np.asarray(wk, dtype=np.float32)
    wv = np.asarray(wv, dtype=np.float32)
    wo = np.asarray(wo, dtype=np.float32)
    slopes = np.asarray(slopes, dtype=np.float32)
    mask = np.asarray(mask, dtype=np.float32)

    B, S, D = x.shape
    HQ = 32
    HD = D // HQ
    n_cores = 8
    HLOC = HQ // n_cores

    causal, zeros, n_lo, n_hi = _analyze_mask(mask[0, 0], S)
    cfg = dict(B=B, S=S, D=D, HLOC=HLOC, HD=HD, MC=512,
               causal=causal, generic_mask=not (causal or zeros),
               n_lo=n_lo, n_hi=n_hi)

    nc = build_program(cfg)
    in_maps = [_make_inputs_for_core(c, x, wq, wk, wv, wo, slopes, mask, cfg)
               for c in range(n_cores)]
    res = run_bass_kernel_spmd(nc, in_maps, core_ids=list(range(n_cores)))
    out = np.zeros((B, S, D), np.float32)
    for c in range(n_cores):
        out += res.results[c]["out"]
    return out


if __name__ == "__main__":
    # quick self-test with a tiny config through CoreSim
    pass
